# revision 6
# baseline (speedup 1.0000x reference)
"""Trainium2 Bass kernel for nn_ComplexMultiheadAttention.

Model (B=2, L=4096, E=512, H=8, D=64, W=128):
  qr,qi = query @ qWr.T + qbr, query @ qWi.T + qbi   (same k; v real part only)
  scores = (qr@kr^T + qi@ki^T) / sqrt(D)             per (b, h)
  mask: position i may attend j iff j >= i - W  (no causal mask)
  probs = softmax(scores);  o = probs @ vr
  out_r = o @ oWr.T + obr;  out_i = o @ oWi.T + obi   -> returns (out_r, out_i)

Sharding: 16 (b, h) units over 8 cores -> each core gets one b and two
adjacent heads (hA, hB). Host pre-transposes q/k/v to [E, L] per b, casts
to bf16, and slices per-head weight blocks. Each core computes a partial
[L, 2E] output (out_r | out_i restricted to its heads' contribution);
host sums the 4 partials per b and adds the output biases.

On-chip per core:
  phase 1: Qc_h = [qr_h; qi_h]^T [128, L] (stacked real/imag head dims on
           partitions), same Kc_h; vr^T [128, L] for both heads, then
           PE-transpose to vr [j, dd] blocks (with a ones column for the
           softmax row-sums).
  phase 2: flash-style attention per head over column blocks jb (keys) and
           query quarters (PSUM capacity):  S^T tile = Kc_blk^T @ Qc via
           TensorE (contraction over the 128 stacked dims = re+im dot),
           exp on ScalarE (PSUM->SBUF, scale=1/8), boundary-block mask by
           elementwise 0/1 multiply, then PV accumulation
           O^T[dd|rowsum, i] += [vr_blk | 1]^T @ pT into PSUM.
  phase 3: normalize O^T by broadcasted reciprocal row-sums, then the
           output projection out[l, 0:512|512:1024] = Onorm^T @ [oWr|oWi]
           columns for this core's heads.
"""

import numpy as np
import ml_dtypes
import orjson

import concourse.bass as bass
import concourse.mybir as mybir
import concourse.tile as tile
from concourse.bass_utils import run_bass_kernel_spmd
from concourse.vector_clock import ScopedClock

F32 = mybir.dt.float32
BF16 = mybir.dt.bfloat16
BF = ml_dtypes.bfloat16

B, L, E, H, D, W = 2, 4096, 512, 8, 64, 128
NBLK = L // 128          # 32 j-blocks
NQ = 4                   # query quarters
QCOLS = L // NQ          # 1024
ET = E // 128            # 4 contraction e-tiles
LC = L // 512            # 8 projection l-chunks


# ---------------------------------------------------------------------------
# Workaround: this walrus build rejects instructions carrying >1 sem wait on
# the TileContext tail drain. Spill extra waits onto standalone wait_ge ops.
def _patched_drain_and_barrier(self, tick_clock, wait_clock):
    nc = self.nc
    drain_inst = nc.sync.drain()
    wait_clock.add_sem_waits(
        drain_inst.ins, ScopedClock({None: tick_clock.global_clock})
    )
    si = drain_inst.ins.sync_info
    if si is not None and len(si.on_wait) > 1:
        waits = list(si.on_wait)
        si.on_wait = waits[:1]
        drain_inst.ins.sync_info = si
        id_to_handle = {h.num: h for h in self.sems.allocated().values()}
        for w in waits[1:]:
            nc.sync.wait_ge(id_to_handle[w.id], w.wait_value)
    nc.all_engine_barrier()
    popped = nc._tile_sem_poison_stack.pop()
    assert popped is self._sem_poison
    nc.clear_and_free_semaphores(list(self.sems.allocated().values()))
    nc.all_engine_barrier()


tile.TileContext._drain_and_barrier = _patched_drain_and_barrier


def _split_bir_waits(data, cap=1):
    """This walrus build rejects >cap sem waits on one instruction; hoist
    extras onto wait-only EventSemaphore instructions inserted just before
    (same engine, same stream position -> identical semantics)."""
    n = 0
    for fn in data["functions"]:
        for bb in fn["blocks"]:
            out = []
            for inst in bb["instructions"]:
                si = inst.get("sync_info")
                if si:
                    ws = si.get("on_wait") or []
                    if len(ws) > cap:
                        for w in ws[:-cap]:
                            n += 1
                            out.append({
                                "debug": inst.get("debug", 0),
                                "engine": inst["engine"],
                                "ins": [], "outs": [],
                                "name": f"sw-{n}-{inst['name']}",
                                "opcode": "EventSemaphore",
                                "sync_info": {"on_update": [],
                                              "on_wait": [w]},
                            })
                        si["on_wait"] = ws[-cap:]
                out.append(inst)
            bb["instructions"] = out
    return data
# ---------------------------------------------------------------------------


def build_program():
    nc = bass.Bass("TRN2", target_bir_lowering=False, debug=False)

    qT = nc.dram_tensor("qT", [E, L], BF16, kind="ExternalInput")
    kT = nc.dram_tensor("kT", [E, L], BF16, kind="ExternalInput")
    vT = nc.dram_tensor("vT", [E, L], BF16, kind="ExternalInput")
    WqA = nc.dram_tensor("WqA", [E, 128], BF16, kind="ExternalInput")
    WqB = nc.dram_tensor("WqB", [E, 128], BF16, kind="ExternalInput")
    WkA = nc.dram_tensor("WkA", [E, 128], BF16, kind="ExternalInput")
    WkB = nc.dram_tensor("WkB", [E, 128], BF16, kind="ExternalInput")
    Wv = nc.dram_tensor("Wv", [E, 128], BF16, kind="ExternalInput")
    Wo = nc.dram_tensor("Wo", [128, 2 * E], BF16, kind="ExternalInput")
    bqA = nc.dram_tensor("bqA", [128, 1], F32, kind="ExternalInput")
    bqB = nc.dram_tensor("bqB", [128, 1], F32, kind="ExternalInput")
    bkA = nc.dram_tensor("bkA", [128, 1], F32, kind="ExternalInput")
    bkB = nc.dram_tensor("bkB", [128, 1], F32, kind="ExternalInput")
    bv = nc.dram_tensor("bv", [128, 1], F32, kind="ExternalInput")
    maskmul = nc.dram_tensor("maskmul", [128, 128], BF16, kind="ExternalInput")
    ident = nc.dram_tensor("ident", [128, 128], BF16, kind="ExternalInput")
    out = nc.dram_tensor("out", [L, 2 * E], F32, kind="ExternalOutput")

    with tile.TileContext(nc) as tc:
        with (
            tc.tile_pool(name="persist", bufs=1) as persist,
            tc.tile_pool(name="work", bufs=3) as work,
        ):
            # ---- constants -------------------------------------------------
            w_sb = {}
            for name, t in [("WqA", WqA), ("WqB", WqB), ("WkA", WkA),
                            ("WkB", WkB), ("Wv", Wv)]:
                ws = persist.tile([128, E], BF16, tag=name, name=name)
                # dst[p, et*128 + m] = W[et*128 + p, m]
                nc.sync.dma_start(
                    ws[:, :].rearrange("p (t m) -> p t m", m=128),
                    t.ap().rearrange("(t p) m -> p t m", p=128),
                )
                w_sb[name] = ws
            wo_sb = persist.tile([128, 2 * E], BF16, tag="Wo")
            nc.sync.dma_start(wo_sb[:, :], Wo.ap())
            b_sb = {}
            for name, t in [("bqA", bqA), ("bqB", bqB), ("bkA", bkA),
                            ("bkB", bkB), ("bv", bv)]:
                bs = persist.tile([128, 1], F32, tag=name, name=name)
                nc.sync.dma_start(bs[:, :], t.ap())
                b_sb[name] = bs
            mask_sb = persist.tile([128, 128], BF16, tag="maskmul")
            nc.sync.dma_start(mask_sb[:, :], maskmul.ap())
            id_sb = persist.tile([128, 128], BF16, tag="ident")
            nc.sync.dma_start(id_sb[:, :], ident.ap())
            zero_sb = persist.tile([128, 512], BF16, tag="zero")
            nc.gpsimd.memset(zero_sb[:, :], 0.0)
            ones64 = persist.tile([1, 64], BF16, tag="ones64")
            nc.gpsimd.memset(ones64[:, :], 1.0)

            # ---- persistent activations -----------------------------------
            Qc = {h: persist.tile([128, L], BF16, tag=f"Qc{h}", name=f"Qc{h}") for h in "AB"}
            Kc = {h: persist.tile([128, L], BF16, tag=f"Kc{h}", name=f"Kc{h}") for h in "AB"}
            vrT = persist.tile([128, L], BF16, tag="vrT")
            vr = {h: persist.tile([128, NBLK * 65], BF16, tag=f"vr{h}", name=f"vr{h}")
                  for h in "AB"}
            onorm = persist.tile([128, L], BF16, tag="onorm")

            # ---- phase 1: projections -------------------------------------
            with (
                tc.tile_pool(name="xt", bufs=5) as xt,
                tc.tile_pool(name="pps", bufs=3, space="PSUM") as pps,
                tc.tile_pool(name="tps", bufs=2, space="PSUM") as tps,
            ):
                for src, targets in [
                    (qT, [("WqA", Qc["A"], "bqA"), ("WqB", Qc["B"], "bqB")]),
                    (kT, [("WkA", Kc["A"], "bkA"), ("WkB", Kc["B"], "bkB")]),
                    (vT, [("Wv", vrT, "bv")]),
                ]:
                    xts = []
                    for et in range(ET):
                        x = xt.tile([128, L], BF16, tag="xt")
                        nc.sync.dma_start(
                            x[:, :], src.ap()[et * 128:(et + 1) * 128, :])
                        xts.append(x)
                    for wname, dst, bname in targets:
                        ws = w_sb[wname]
                        for lc in range(LC):
                            ps = pps.tile([128, 512], F32, tag="pps")
                            for et in range(ET):
                                nc.tensor.matmul(
                                    ps[:, :],
                                    ws[:, et * 128:(et + 1) * 128],
                                    xts[et][:, lc * 512:(lc + 1) * 512],
                                    start=(et == 0), stop=(et == ET - 1),
                                )
                            nc.scalar.activation(
                                dst[:, lc * 512:(lc + 1) * 512], ps[:, :],
                                mybir.ActivationFunctionType.Identity,
                                bias=b_sb[bname][:, :],
                            )

                # vr^T -> vr blocks (with ones column at dd=64)
                for h in "AB":
                    ones_ap = vr[h][:, :].rearrange(
                        "p (k c) -> p k c", c=65)[:, :, 64:65]
                    nc.gpsimd.memset(ones_ap, 1.0)
                for jb in range(NBLK):
                    tp = tps.tile([128, 128], BF16, tag="tps")
                    nc.tensor.transpose(
                        tp[:, :], vrT[:, jb * 128:(jb + 1) * 128], id_sb[:, :])
                    nc.vector.tensor_copy(
                        vr["A"][:, jb * 65:jb * 65 + 64], tp[:, 0:64])
                    nc.vector.tensor_copy(
                        vr["B"][:, jb * 65:jb * 65 + 64], tp[:, 64:128])

            # ---- phase 2: attention ---------------------------------------
            with (
                tc.tile_pool(name="sps", bufs=2, space="PSUM") as sps,
                tc.tile_pool(name="ops", bufs=2, space="PSUM") as ops,
                tc.tile_pool(name="ptp", bufs=3) as ptp,
                tc.tile_pool(name="nrm", bufs=2) as nrm,
            ):
                for hi, h in enumerate("AB"):
                    qch, kch, vrh = Qc[h], Kc[h], vr[h]
                    for q in range(NQ):
                        qs = q * QCOLS
                        qn = qs // 128
                        jb_min = max(0, qn - 1)
                        oacc = ops.tile([65, QCOLS], F32, tag="oacc")
                        # zero the accumulator via matmuls (sets has_written)
                        for n0 in range(0, QCOLS, 512):
                            nc.tensor.matmul(
                                oacc[:, n0:n0 + 512], vrh[:, 0:65],
                                zero_sb[:, :], start=True, stop=False,
                                skip_group_check=True,
                            )
                        prev = None  # (pt tile, ic_hi, jb)
                        for jb in range(jb_min, NBLK):
                            ic_hi = min(QCOLS, (jb + 2) * 128 - qs)
                            st = sps.tile([128, QCOLS], F32, tag="st")
                            for n0 in range(0, ic_hi, 512):
                                nn = min(512, ic_hi - n0)
                                nc.tensor.matmul(
                                    st[:, n0:n0 + nn],
                                    kch[:, jb * 128:(jb + 1) * 128],
                                    qch[:, qs + n0:qs + n0 + nn],
                                    start=True, stop=True,
                                    skip_group_check=True,
                                )
                            pt = ptp.tile([128, QCOLS], BF16, tag="pt")
                            nc.scalar.activation(
                                pt[:, 0:ic_hi], st[:, 0:ic_hi],
                                mybir.ActivationFunctionType.Exp, scale=0.125)
                            if ic_hi == (jb + 2) * 128 - qs:
                                # boundary i-block: zero disallowed (jj < ii)
                                nc.vector.tensor_mul(
                                    pt[:, ic_hi - 128:ic_hi],
                                    pt[:, ic_hi - 128:ic_hi], mask_sb[:, :])
                            if prev is not None:
                                ppt, pic, pjb = prev
                                for n0 in range(0, pic, 512):
                                    nn = min(512, pic - n0)
                                    nc.tensor.matmul(
                                        oacc[:, n0:n0 + nn],
                                        vrh[:, pjb * 65:(pjb + 1) * 65],
                                        ppt[:, n0:n0 + nn],
                                        start=False, stop=False,
                                        skip_group_check=True,
                                    )
                            prev = (pt, ic_hi, jb)
                        ppt, pic, pjb = prev
                        for n0 in range(0, pic, 512):
                            nn = min(512, pic - n0)
                            nc.tensor.matmul(
                                oacc[:, n0:n0 + nn],
                                vrh[:, pjb * 65:(pjb + 1) * 65],
                                ppt[:, n0:n0 + nn],
                                start=False, stop=True,
                                skip_group_check=True,
                            )
                        # normalize: Onorm rows [64*hi, 64*hi+64)
                        rs = nrm.tile([1, QCOLS], BF16, tag="rs")
                        nc.vector.tensor_copy(rs[:, :], oacc[64:65, :])
                        bcp = sps.tile([64, QCOLS], F32, tag="st", name="bcp")
                        for n0 in range(0, QCOLS, 512):
                            nc.tensor.matmul(
                                bcp[:, n0:n0 + 512], ones64[:, :],
                                rs[:, n0:n0 + 512],
                                start=True, stop=True, skip_group_check=True,
                            )
                        bc = nrm.tile([64, QCOLS], F32, tag="bc")
                        nc.vector.reciprocal(bc[:, :], bcp[:, :])
                        nc.vector.tensor_mul(
                            onorm[64 * hi:64 * hi + 64, qs:qs + QCOLS],
                            oacc[0:64, :], bc[:, :])

            # ---- phase 3: output projection -------------------------------
            with tc.tile_pool(name="fps", bufs=2, space="PSUM") as fps:
                for lt in range(NBLK):
                    fp = fps.tile([128, 2 * E], F32, tag="fps")
                    for n0 in range(0, 2 * E, 512):
                        nc.tensor.matmul(
                            fp[:, n0:n0 + 512],
                            onorm[:, lt * 128:(lt + 1) * 128],
                            wo_sb[:, n0:n0 + 512],
                            start=True, stop=True, skip_group_check=True,
                        )
                    os = work.tile([128, 2 * E], F32, tag="osb")
                    if lt % 2 == 0:
                        nc.scalar.copy(os[:, :], fp[:, :])
                    else:
                        nc.vector.tensor_copy(os[:, :], fp[:, :])
                    nc.sync.dma_start(
                        out.ap()[lt * 128:(lt + 1) * 128, :], os[:, :])

    _orig_to_json = nc.to_json_bytes

    def _to_json_bytes_split():
        return orjson.dumps(_split_bir_waits(orjson.loads(_orig_to_json())))

    nc.to_json_bytes = _to_json_bytes_split
    return nc


def shard_inputs(inputs):
    """Build the 8 per-core input maps (host-side layout prep)."""
    q, k, v = inputs["query"], inputs["key"], inputs["value"]
    qWr, qWi = np.asarray(inputs["qWr"]), np.asarray(inputs["qWi"])
    kWr, kWi = np.asarray(inputs["kWr"]), np.asarray(inputs["kWi"])
    vWr = np.asarray(inputs["vWr"])
    oWr, oWi = np.asarray(inputs["oWr"]), np.asarray(inputs["oWi"])
    qbr, qbi = np.asarray(inputs["qbr"]), np.asarray(inputs["qbi"])
    kbr, kbi = np.asarray(inputs["kbr"]), np.asarray(inputs["kbi"])
    vbr = np.asarray(inputs["vbr"])

    mask = np.tril(np.ones((128, 128), np.float32)).astype(BF)  # jj >= ii
    ident = np.eye(128, dtype=np.float32).astype(BF)

    xT = {}
    for b in range(B):
        xT[b] = tuple(
            np.ascontiguousarray(np.asarray(t)[b].T).astype(BF)
            for t in (q, k, v)
        )

    def wq(Wr, Wi, h):
        return np.ascontiguousarray(
            np.concatenate([Wr[h * D:(h + 1) * D], Wi[h * D:(h + 1) * D]], 0).T
        ).astype(BF)

    def bias2(br, bi, h):
        return np.concatenate(
            [br[h * D:(h + 1) * D], bi[h * D:(h + 1) * D]]
        ).astype(np.float32)[:, None]

    in_maps = []
    for c in range(8):
        b = c // 4
        hA = 2 * (c % 4)
        hB = hA + 1
        qTb, kTb, vTb = xT[b]
        wv = np.ascontiguousarray(np.concatenate(
            [vWr[hA * D:(hA + 1) * D], vWr[hB * D:(hB + 1) * D]], 0).T
        ).astype(BF)
        wo_r = np.concatenate(
            [oWr[:, hA * D:(hA + 1) * D], oWr[:, hB * D:(hB + 1) * D]], 1).T
        wo_i = np.concatenate(
            [oWi[:, hA * D:(hA + 1) * D], oWi[:, hB * D:(hB + 1) * D]], 1).T
        wo = np.ascontiguousarray(
            np.concatenate([wo_r, wo_i], 1)).astype(BF)
        bvc = np.concatenate(
            [vbr[hA * D:(hA + 1) * D], vbr[hB * D:(hB + 1) * D]]
        ).astype(np.float32)[:, None]
        in_maps.append({
            "qT": qTb, "kT": kTb, "vT": vTb,
            "WqA": wq(qWr, qWi, hA), "WqB": wq(qWr, qWi, hB),
            "WkA": wq(kWr, kWi, hA), "WkB": wq(kWr, kWi, hB),
            "Wv": wv, "Wo": wo,
            "bqA": bias2(qbr, qbi, hA), "bqB": bias2(qbr, qbi, hB),
            "bkA": bias2(kbr, kbi, hA), "bkB": bias2(kbr, kbi, hB),
            "bv": bvc,
            "maskmul": mask, "ident": ident,
        })
    return in_maps


_NC_CACHE = None


def kernel(**inputs):
    global _NC_CACHE
    if _NC_CACHE is None:
        _NC_CACHE = build_program()
    nc = _NC_CACHE
    in_maps = shard_inputs(inputs)
    res = run_bass_kernel_spmd(nc, in_maps, core_ids=list(range(8)))
    obr = np.asarray(inputs["obr"], np.float32)
    obi = np.asarray(inputs["obi"], np.float32)
    acc = np.zeros((B, L, 2 * E), np.float32)
    for c in range(8):
        acc[c // 4] += np.asarray(res.results[c]["out"], np.float32)
    out_r = acc[:, :, :E] + obr
    out_i = acc[:, :, E:] + obi
    return out_r, out_i


# revision 8
# speedup vs baseline: 20929.1971x; 20929.1971x over previous
"""Trainium2 Bass kernel for nn_ComplexMultiheadAttention.

Model (B=2, L=4096, E=512, H=8, D=64, W=128):
  qr,qi = query @ qWr.T + qbr, query @ qWi.T + qbi   (same k; v real part only)
  scores = (qr@kr^T + qi@ki^T) / sqrt(D)             per (b, h)
  mask: position i may attend j iff j >= i - W  (no causal mask)
  probs = softmax(scores);  o = probs @ vr
  out_r = o @ oWr.T + obr;  out_i = o @ oWi.T + obi   -> returns (out_r, out_i)

Sharding: 16 (b, h) units over 8 cores -> each core gets one b and two
adjacent heads (hA, hB). Host pre-transposes q/k/v to [E, L] per b, casts
to bf16, and slices per-head weight blocks. Each core computes a partial
[L, 2E] output (out_r | out_i restricted to its heads' contribution);
host sums the 4 partials per b and adds the output biases.

On-chip per core:
  phase 1: Qc_h = [qr_h; qi_h]^T [128, L] (stacked real/imag head dims on
           partitions), same Kc_h; vr^T [128, L] for both heads, then
           PE-transpose to vr [j, dd] blocks (with a ones column for the
           softmax row-sums).
  phase 2: flash-style attention per head over column blocks jb (keys) and
           query quarters (PSUM capacity):  S^T tile = Kc_blk^T @ Qc via
           TensorE (contraction over the 128 stacked dims = re+im dot),
           exp on ScalarE (PSUM->SBUF, scale=1/8), boundary-block mask by
           elementwise 0/1 multiply, then PV accumulation
           O^T[dd|rowsum, i] += [vr_blk | 1]^T @ pT into PSUM.
  phase 3: normalize O^T by broadcasted reciprocal row-sums, then the
           output projection out[l, 0:512|512:1024] = Onorm^T @ [oWr|oWi]
           columns for this core's heads.
"""

import numpy as np
import ml_dtypes
import orjson

import concourse.bass as bass
import concourse.mybir as mybir
import concourse.tile as tile
from concourse.bass_utils import run_bass_kernel_spmd
from concourse.vector_clock import ScopedClock

F32 = mybir.dt.float32
BF16 = mybir.dt.bfloat16
BF = ml_dtypes.bfloat16

B, L, E, H, D, W = 2, 4096, 512, 8, 64, 128
NBLK = L // 128          # 32 j-blocks
NQ = 4                   # query quarters
QCOLS = L // NQ          # 1024
ET = E // 128            # 4 contraction e-tiles
LC = L // 512            # 8 projection l-chunks


# ---------------------------------------------------------------------------
# Workaround: this walrus build rejects instructions carrying >1 sem wait on
# the TileContext tail drain. Spill extra waits onto standalone wait_ge ops.
def _patched_drain_and_barrier(self, tick_clock, wait_clock):
    nc = self.nc
    drain_inst = nc.sync.drain()
    wait_clock.add_sem_waits(
        drain_inst.ins, ScopedClock({None: tick_clock.global_clock})
    )
    si = drain_inst.ins.sync_info
    if si is not None and len(si.on_wait) > 1:
        waits = list(si.on_wait)
        si.on_wait = waits[:1]
        drain_inst.ins.sync_info = si
        id_to_handle = {h.num: h for h in self.sems.allocated().values()}
        for w in waits[1:]:
            nc.sync.wait_ge(id_to_handle[w.id], w.wait_value)
    nc.all_engine_barrier()
    popped = nc._tile_sem_poison_stack.pop()
    assert popped is self._sem_poison
    nc.clear_and_free_semaphores(list(self.sems.allocated().values()))
    nc.all_engine_barrier()


tile.TileContext._drain_and_barrier = _patched_drain_and_barrier


def _split_bir_waits(data, cap=1):
    """This walrus build rejects >cap sem waits on one instruction; hoist
    extras onto wait-only EventSemaphore instructions inserted just before
    (same engine, same stream position -> identical semantics)."""
    n = 0
    for fn in data["functions"]:
        for bb in fn["blocks"]:
            out = []
            for inst in bb["instructions"]:
                si = inst.get("sync_info")
                if si:
                    ws = si.get("on_wait") or []
                    if len(ws) > cap:
                        for w in ws[:-cap]:
                            n += 1
                            out.append({
                                "debug": inst.get("debug", 0),
                                "engine": inst["engine"],
                                "ins": [], "outs": [],
                                "name": f"sw-{n}-{inst['name']}",
                                "opcode": "EventSemaphore",
                                "sync_info": {"on_update": [],
                                              "on_wait": [w]},
                            })
                        si["on_wait"] = ws[-cap:]
                out.append(inst)
            bb["instructions"] = out
    return data
# ---------------------------------------------------------------------------


def build_program():
    nc = bass.Bass("TRN2", target_bir_lowering=False, debug=False)

    qT = nc.dram_tensor("qT", [E, L], BF16, kind="ExternalInput")
    kT = nc.dram_tensor("kT", [E, L], BF16, kind="ExternalInput")
    vT = nc.dram_tensor("vT", [E, L], BF16, kind="ExternalInput")
    WqA = nc.dram_tensor("WqA", [E, 128], BF16, kind="ExternalInput")
    WqB = nc.dram_tensor("WqB", [E, 128], BF16, kind="ExternalInput")
    WkA = nc.dram_tensor("WkA", [E, 128], BF16, kind="ExternalInput")
    WkB = nc.dram_tensor("WkB", [E, 128], BF16, kind="ExternalInput")
    Wv = nc.dram_tensor("Wv", [E, 128], BF16, kind="ExternalInput")
    Wo = nc.dram_tensor("Wo", [128, 2 * E], BF16, kind="ExternalInput")
    bqA = nc.dram_tensor("bqA", [128, 1], F32, kind="ExternalInput")
    bqB = nc.dram_tensor("bqB", [128, 1], F32, kind="ExternalInput")
    bkA = nc.dram_tensor("bkA", [128, 1], F32, kind="ExternalInput")
    bkB = nc.dram_tensor("bkB", [128, 1], F32, kind="ExternalInput")
    bv = nc.dram_tensor("bv", [128, 1], F32, kind="ExternalInput")
    maskmul = nc.dram_tensor("maskmul", [128, 128], BF16, kind="ExternalInput")
    ident = nc.dram_tensor("ident", [128, 128], BF16, kind="ExternalInput")
    out = nc.dram_tensor("out", [L, 2 * E], F32, kind="ExternalOutput")

    with tile.TileContext(nc) as tc:
        with (
            tc.tile_pool(name="persist", bufs=1) as persist,
            tc.tile_pool(name="work", bufs=3) as work,
        ):
            # ---- constants -------------------------------------------------
            w_sb = {}
            for name, t in [("WqA", WqA), ("WqB", WqB), ("WkA", WkA),
                            ("WkB", WkB), ("Wv", Wv)]:
                ws = persist.tile([128, E], BF16, tag=name, name=name)
                # dst[p, et*128 + m] = W[et*128 + p, m]
                nc.sync.dma_start(
                    ws[:, :].rearrange("p (t m) -> p t m", m=128),
                    t.ap().rearrange("(t p) m -> p t m", p=128),
                )
                w_sb[name] = ws
            wo_sb = persist.tile([128, 2 * E], BF16, tag="Wo")
            nc.sync.dma_start(wo_sb[:, :], Wo.ap())
            b_sb = {}
            for name, t in [("bqA", bqA), ("bqB", bqB), ("bkA", bkA),
                            ("bkB", bkB), ("bv", bv)]:
                bs = persist.tile([128, 1], F32, tag=name, name=name)
                nc.sync.dma_start(bs[:, :], t.ap())
                b_sb[name] = bs
            mask_sb = persist.tile([128, 128], BF16, tag="maskmul")
            nc.sync.dma_start(mask_sb[:, :], maskmul.ap())
            id_sb = persist.tile([128, 128], BF16, tag="ident")
            nc.sync.dma_start(id_sb[:, :], ident.ap())
            zero_sb = persist.tile([128, 512], BF16, tag="zero")
            nc.gpsimd.memset(zero_sb[:, :], 0.0)
            ones64 = persist.tile([1, 64], BF16, tag="ones64")
            nc.gpsimd.memset(ones64[:, :], 1.0)

            # ---- persistent activations -----------------------------------
            Qc = {h: persist.tile([128, L], BF16, tag=f"Qc{h}", name=f"Qc{h}") for h in "AB"}
            Kc = {h: persist.tile([128, L], BF16, tag=f"Kc{h}", name=f"Kc{h}") for h in "AB"}
            vrT = persist.tile([128, L], BF16, tag="vrT")
            vr = {h: persist.tile([128, NBLK * 65], BF16, tag=f"vr{h}", name=f"vr{h}")
                  for h in "AB"}
            onorm = persist.tile([128, L], BF16, tag="onorm")

            # ---- phase 1: projections -------------------------------------
            with (
                tc.tile_pool(name="xt", bufs=5) as xt,
                tc.tile_pool(name="pps", bufs=3, space="PSUM") as pps,
                tc.tile_pool(name="tps", bufs=2, space="PSUM") as tps,
            ):
                for src, targets in [
                    (qT, [("WqA", Qc["A"], "bqA"), ("WqB", Qc["B"], "bqB")]),
                    (kT, [("WkA", Kc["A"], "bkA"), ("WkB", Kc["B"], "bkB")]),
                    (vT, [("Wv", vrT, "bv")]),
                ]:
                    xts = []
                    for et in range(ET):
                        x = xt.tile([128, L], BF16, tag="xt")
                        nc.sync.dma_start(
                            x[:, :], src.ap()[et * 128:(et + 1) * 128, :])
                        xts.append(x)
                    for wname, dst, bname in targets:
                        ws = w_sb[wname]
                        for lc in range(LC):
                            ps = pps.tile([128, 512], F32, tag="pps")
                            for et in range(ET):
                                nc.tensor.matmul(
                                    ps[:, :],
                                    ws[:, et * 128:(et + 1) * 128],
                                    xts[et][:, lc * 512:(lc + 1) * 512],
                                    start=(et == 0), stop=(et == ET - 1),
                                )
                            nc.scalar.activation(
                                dst[:, lc * 512:(lc + 1) * 512], ps[:, :],
                                mybir.ActivationFunctionType.Identity,
                                bias=b_sb[bname][:, :],
                            )

                # vr^T -> vr blocks (with ones column at dd=64)
                for h in "AB":
                    ones_ap = vr[h][:, :].rearrange(
                        "p (k c) -> p k c", c=65)[:, :, 64:65]
                    nc.gpsimd.memset(ones_ap, 1.0)
                for jb in range(NBLK):
                    tp = tps.tile([128, 128], BF16, tag="tps")
                    nc.tensor.transpose(
                        tp[:, :], vrT[:, jb * 128:(jb + 1) * 128], id_sb[:, :])
                    nc.vector.tensor_copy(
                        vr["A"][:, jb * 65:jb * 65 + 64], tp[:, 0:64])
                    nc.vector.tensor_copy(
                        vr["B"][:, jb * 65:jb * 65 + 64], tp[:, 64:128])

            # ---- phase 2: attention ---------------------------------------
            with (
                tc.tile_pool(name="sps", bufs=2, space="PSUM") as sps,
                tc.tile_pool(name="ops", bufs=2, space="PSUM") as ops,
                tc.tile_pool(name="ptp", bufs=3) as ptp,
                tc.tile_pool(name="nrm", bufs=2) as nrm,
            ):
                for hi, h in enumerate("AB"):
                    qch, kch, vrh = Qc[h], Kc[h], vr[h]
                    for q in range(NQ):
                        qs = q * QCOLS
                        qn = qs // 128
                        jb_min = max(0, qn - 1)
                        oacc = ops.tile([65, QCOLS], F32, tag="oacc")
                        # zero the accumulator via matmuls (sets has_written)
                        for n0 in range(0, QCOLS, 512):
                            nc.tensor.matmul(
                                oacc[:, n0:n0 + 512], vrh[:, 0:65],
                                zero_sb[:, :], start=True, stop=False,
                                skip_group_check=True,
                            )
                        prev = None  # (pt tile, ic_hi, jb)
                        for jb in range(jb_min, NBLK):
                            ic_hi = min(QCOLS, (jb + 2) * 128 - qs)
                            st = sps.tile([128, QCOLS], F32, tag="st")
                            for n0 in range(0, ic_hi, 512):
                                nn = min(512, ic_hi - n0)
                                nc.tensor.matmul(
                                    st[:, n0:n0 + nn],
                                    kch[:, jb * 128:(jb + 1) * 128],
                                    qch[:, qs + n0:qs + n0 + nn],
                                    start=True, stop=True,
                                    skip_group_check=True,
                                )
                            pt = ptp.tile([128, QCOLS], BF16, tag="pt")
                            nc.scalar.activation(
                                pt[:, 0:ic_hi], st[:, 0:ic_hi],
                                mybir.ActivationFunctionType.Exp, scale=0.125)
                            if ic_hi == (jb + 2) * 128 - qs:
                                # boundary i-block: zero disallowed (jj < ii)
                                nc.vector.tensor_mul(
                                    pt[:, ic_hi - 128:ic_hi],
                                    pt[:, ic_hi - 128:ic_hi], mask_sb[:, :])
                            if prev is not None:
                                ppt, pic, pjb = prev
                                for n0 in range(0, pic, 512):
                                    nn = min(512, pic - n0)
                                    nc.tensor.matmul(
                                        oacc[:, n0:n0 + nn],
                                        vrh[:, pjb * 65:(pjb + 1) * 65],
                                        ppt[:, n0:n0 + nn],
                                        start=False, stop=False,
                                        skip_group_check=True,
                                    )
                            prev = (pt, ic_hi, jb)
                        ppt, pic, pjb = prev
                        for n0 in range(0, pic, 512):
                            nn = min(512, pic - n0)
                            nc.tensor.matmul(
                                oacc[:, n0:n0 + nn],
                                vrh[:, pjb * 65:(pjb + 1) * 65],
                                ppt[:, n0:n0 + nn],
                                start=False, stop=True,
                                skip_group_check=True,
                            )
                        # normalize: Onorm rows [64*hi, 64*hi+64)
                        rs = nrm.tile([1, QCOLS], BF16, tag="rs")
                        nc.vector.tensor_copy(rs[:, :], oacc[64:65, :])
                        bcp = sps.tile([64, QCOLS], F32, tag="st", name="bcp")
                        for n0 in range(0, QCOLS, 512):
                            nc.tensor.matmul(
                                bcp[:, n0:n0 + 512], ones64[:, :],
                                rs[:, n0:n0 + 512],
                                start=True, stop=True, skip_group_check=True,
                            )
                        bc = nrm.tile([64, QCOLS], F32, tag="bc")
                        nc.vector.reciprocal(bc[:, :], bcp[:, :])
                        nc.vector.tensor_mul(
                            onorm[64 * hi:64 * hi + 64, qs:qs + QCOLS],
                            oacc[0:64, :], bc[:, :])

            # ---- phase 3: output projection -------------------------------
            with tc.tile_pool(name="fps", bufs=2, space="PSUM") as fps:
                for lt in range(NBLK):
                    fp = fps.tile([128, 2 * E], F32, tag="fps")
                    for n0 in range(0, 2 * E, 512):
                        nc.tensor.matmul(
                            fp[:, n0:n0 + 512],
                            onorm[:, lt * 128:(lt + 1) * 128],
                            wo_sb[:, n0:n0 + 512],
                            start=True, stop=True, skip_group_check=True,
                        )
                    os = work.tile([128, 2 * E], F32, tag="osb")
                    if lt % 2 == 0:
                        nc.scalar.copy(os[:, :], fp[:, :])
                    else:
                        nc.vector.tensor_copy(os[:, :], fp[:, :])
                    nc.sync.dma_start(
                        out.ap()[lt * 128:(lt + 1) * 128, :], os[:, :])

    _orig_to_json = nc.to_json_bytes

    def _to_json_bytes_split():
        return orjson.dumps(_split_bir_waits(orjson.loads(_orig_to_json())))

    nc.to_json_bytes = _to_json_bytes_split
    return nc


def shard_inputs(inputs):
    """Build the 8 per-core input maps (host-side layout prep)."""
    q, k, v = inputs["query"], inputs["key"], inputs["value"]
    qWr, qWi = np.asarray(inputs["qWr"]), np.asarray(inputs["qWi"])
    kWr, kWi = np.asarray(inputs["kWr"]), np.asarray(inputs["kWi"])
    vWr = np.asarray(inputs["vWr"])
    oWr, oWi = np.asarray(inputs["oWr"]), np.asarray(inputs["oWi"])
    qbr, qbi = np.asarray(inputs["qbr"]), np.asarray(inputs["qbi"])
    kbr, kbi = np.asarray(inputs["kbr"]), np.asarray(inputs["kbi"])
    vbr = np.asarray(inputs["vbr"])

    mask = np.tril(np.ones((128, 128), np.float32)).astype(BF)  # jj >= ii
    ident = np.eye(128, dtype=np.float32).astype(BF)

    xT = {}
    for b in range(B):
        xT[b] = tuple(
            np.ascontiguousarray(np.asarray(t)[b].T).astype(BF)
            for t in (q, k, v)
        )

    def wq(Wr, Wi, h):
        return np.ascontiguousarray(
            np.concatenate([Wr[h * D:(h + 1) * D], Wi[h * D:(h + 1) * D]], 0).T
        ).astype(BF)

    def bias2(br, bi, h):
        return np.concatenate(
            [br[h * D:(h + 1) * D], bi[h * D:(h + 1) * D]]
        ).astype(np.float32)[:, None]

    in_maps = []
    for c in range(8):
        b = c // 4
        hA = 2 * (c % 4)
        hB = hA + 1
        qTb, kTb, vTb = xT[b]
        wv = np.ascontiguousarray(np.concatenate(
            [vWr[hA * D:(hA + 1) * D], vWr[hB * D:(hB + 1) * D]], 0).T
        ).astype(BF)
        wo_r = np.concatenate(
            [oWr[:, hA * D:(hA + 1) * D], oWr[:, hB * D:(hB + 1) * D]], 1).T
        wo_i = np.concatenate(
            [oWi[:, hA * D:(hA + 1) * D], oWi[:, hB * D:(hB + 1) * D]], 1).T
        wo = np.ascontiguousarray(
            np.concatenate([wo_r, wo_i], 1)).astype(BF)
        bvc = np.concatenate(
            [vbr[hA * D:(hA + 1) * D], vbr[hB * D:(hB + 1) * D]]
        ).astype(np.float32)[:, None]
        in_maps.append({
            "qT": qTb, "kT": kTb, "vT": vTb,
            "WqA": wq(qWr, qWi, hA), "WqB": wq(qWr, qWi, hB),
            "WkA": wq(kWr, kWi, hA), "WkB": wq(kWr, kWi, hB),
            "Wv": wv, "Wo": wo,
            "bqA": bias2(qbr, qbi, hA), "bqB": bias2(qbr, qbi, hB),
            "bkA": bias2(kbr, kbi, hA), "bkB": bias2(kbr, kbi, hB),
            "bv": bvc,
            "maskmul": mask, "ident": ident,
        })
    return in_maps


_NC_CACHE = None


def kernel(**inputs):
    global _NC_CACHE
    if _NC_CACHE is None:
        _NC_CACHE = build_program()
    nc = _NC_CACHE
    in_maps = shard_inputs(inputs)
    res = run_bass_kernel_spmd(nc, in_maps, core_ids=list(range(8)))
    obr = np.asarray(inputs["obr"], np.float32)
    obi = np.asarray(inputs["obi"], np.float32)
    acc = np.zeros((B, L, 2 * E), np.float32)
    for c in range(8):
        acc[c // 4] += np.asarray(res.results[c]["out"], np.float32)
    out_r = acc[:, :, :E] + obr
    out_i = acc[:, :, E:] + obi
    return out_r, out_i
